# revision 9
# baseline (speedup 1.0000x reference)
"""Trainium2 Bass kernel for AttentionReadout2DPDE.

Reference computation (per sample b):
    hid  = relu(measurement @ W1 + b1)                       [B, H]
    raw  = (hid @ W2 + b2).reshape(B, Q, 2 + D)
    xy   = sigmoid(raw[:, :, :2])                            [B, Q, 2]
    w    = raw[:, :, 2:]                                     [B, Q, D]
    mu, sd = mean/std(field_u[b])  (std unbiased, clamp 1e-6)
    pde  = bilinear_sample((field_u - mu) / sd, xy)          [B, Q]
    out  = einsum('bq,bqd->bd', pde, w)                      [B, D]

Key fact used: bilinear weights sum to 1, so
    bilinear(field_norm) = (bilinear(field_u) - mu) / sd
and the normalized field never needs to be materialized.  The kernel
streams each sample's field once (sum on VectorE, sum-of-squares on
ScalarE via activation accum), gathers the 4 bilinear corners per query
with an indirect DMA, and applies the normalization to the 64 sampled
values only.

Sharding: pure data parallel, batch 256 -> 8 cores x 32 samples.
"""

import numpy as np
from contextlib import ExitStack

import concourse.bass as bass
import concourse.tile as tile
import concourse.mybir as mybir
from concourse import bacc
from concourse.bass_utils import run_bass_kernel_spmd
from concourse.masks import make_identity

F32 = mybir.dt.float32
I32 = mybir.dt.int32
AF = mybir.ActivationFunctionType
OP = mybir.AluOpType
AX = mybir.AxisListType

B, S, NX, NY = 256, 256, 512, 512
Q, D, H = 64, 32, 256
CH = 2 + D            # 34 channels per query
NCORES = 8
BL = B // NCORES      # 32 samples per core
FS = NX * NY          # 262144 field elems per sample
P = 128
COLS = FS // P        # 2048 field elems per partition per sample
SPD = 2               # samples per field DMA (2 MB transfers)

DBG = False           # extra "dbg" output tensor with intermediates
DBGW = 512


def _body(ctx: ExitStack, tc: "tile.TileContext", meas_d, field_d, w1_d, b1_d,
          w2_d, b2_d, bbase_d, out_d, dbg_d):
    nc = tc.nc
    const = ctx.enter_context(tc.tile_pool(name="const", bufs=1))
    spool = ctx.enter_context(tc.tile_pool(name="small", bufs=1))
    fpool = ctx.enter_context(tc.tile_pool(name="field", bufs=3))
    scr = ctx.enter_context(tc.tile_pool(name="scratch", bufs=1))
    psum = ctx.enter_context(tc.tile_pool(name="psum", bufs=3, space="PSUM"))

    # ---------------- constants / weights (SWDGE queue; HWDGE stays free
    # for the field stream) ----------------
    w1_sb = const.tile([P, 2, H], F32)
    w2_sb = const.tile([P, 2, Q * CH], F32)
    b1_sb = const.tile([P, 2], F32)
    b2_sb = const.tile([1, Q * CH], F32)
    meas_sb = const.tile([BL, S], F32)
    bbase_sb = const.tile([Q, BL], F32)
    ident = const.tile([P, P], F32)
    ones1 = const.tile([1, Q], F32)
    for k in range(2):
        nc.gpsimd.dma_start(out=w1_sb[:, k, :], in_=w1_d[k * P:(k + 1) * P, :])
        nc.gpsimd.dma_start(out=w2_sb[:, k, :], in_=w2_d[k * P:(k + 1) * P, :])
        nc.gpsimd.dma_start(out=b1_sb[:, k:k + 1],
                            in_=b1_d[k * P:(k + 1) * P, None])
    nc.gpsimd.dma_start(out=b2_sb[:], in_=b2_d[None, :])
    nc.gpsimd.dma_start(out=meas_sb[:], in_=meas_d[:])
    nc.gpsimd.dma_start(out=bbase_sb[:], in_=bbase_d[:])
    make_identity(nc, ident[:])
    nc.gpsimd.memset(ones1[:], 1.0)

    # ---------------- MLP ----------------
    # measT[s, b] via PE transpose (two 32x128 -> 128x32 chunks)
    measT_sb = spool.tile([P, 2, BL], F32)
    for k in range(2):
        mt_ps = psum.tile([P, BL], F32, tag="mm")
        nc.tensor.transpose(out=mt_ps[:], in_=meas_sb[:, k * P:(k + 1) * P],
                            identity=ident[0:BL, 0:BL])
        nc.vector.tensor_copy(out=measT_sb[:, k, :], in_=mt_ps[:])

    # hidT[h, b] = relu(W1.T @ measT + b1)
    hidT_sb = spool.tile([P, 2, BL], F32)
    for hk in range(2):
        h_ps = psum.tile([P, BL], F32, tag="mm")
        for sk in range(2):
            nc.tensor.matmul(out=h_ps[:],
                             lhsT=w1_sb[:, sk, hk * P:(hk + 1) * P],
                             rhs=measT_sb[:, sk, :],
                             start=(sk == 0), stop=(sk == 1))
        nc.scalar.activation(out=hidT_sb[:, hk, :], in_=h_ps[:], func=AF.Relu,
                             bias=b1_sb[:, hk:hk + 1], scale=1.0)

    # raw[b, q*34+c] = hid @ W2 + b2   (bias folded in as a k=1 matmul)
    raw_sb = spool.tile([BL, Q * CH], F32)
    for off in range(0, Q * CH, 512):
        nsz = min(512, Q * CH - off)
        r_ps = psum.tile([BL, nsz], F32, tag="mm")
        for hk in range(2):
            nc.tensor.matmul(out=r_ps[:], lhsT=hidT_sb[:, hk, :],
                             rhs=w2_sb[:, hk, off:off + nsz],
                             start=(hk == 0), stop=False)
        nc.tensor.matmul(out=r_ps[:], lhsT=ones1[:, 0:BL],
                         rhs=b2_sb[:, off:off + nsz], start=False, stop=True)
        nc.vector.tensor_copy(out=raw_sb[:, off:off + nsz], in_=r_ps[:])

    qv = raw_sb[:].rearrange("p (q c) -> p q c", c=CH)

    # ---------------- query positions, q-on-partition layout ------------
    # rawT_x[q, b] / rawT_y[q, b] via strided-lhsT matmuls picking the
    # c=0 / c=1 channel columns of W2; bias added as a k=1 matmul with
    # lhsT = b2 channel view, rhs = ones.
    w2v = [w2_sb[:, hk, :].rearrange("p (q c) -> p q c", c=CH)
           for hk in range(2)]
    b2v = b2_sb[:].rearrange("o (q c) -> o q c", c=CH)
    pxt = {}
    for ci, name in ((0, "x"), (1, "y")):
        ps = psum.tile([Q, BL], F32, tag="mm")
        for hk in range(2):
            nc.tensor.matmul(out=ps[:],
                             lhsT=w2v[hk][:, :, ci:ci + 1],
                             rhs=hidT_sb[:, hk, :],
                             start=(hk == 0), stop=False)
        nc.tensor.matmul(out=ps[:],
                         lhsT=b2v[:, :, ci:ci + 1].rearrange("o q c -> o (q c)"),
                         rhs=ones1[:, 0:BL], start=False, stop=True)
        sg = spool.tile([Q, BL], F32, tag=f"sig{name}")
        nc.scalar.activation(out=sg[:], in_=ps[:], func=AF.Sigmoid)
        p = spool.tile([Q, BL], F32, tag=f"p{name}")
        nc.vector.tensor_scalar_mul(out=p[:], in0=sg[:], scalar1=float(NY - 1))
        pxt[name] = p

    # floor via the 2^23 magic-number round + is_gt fixup (exact for
    # 0 <= p < 2^22; no dependence on any int-cast rounding mode):
    #   rnd = round_nearest(p); v0 = rnd - (rnd > p); clamp to [0, 510]
    MAGIC = 8388608.0
    pos0 = {}
    wgt = {}
    for name in ("x", "y"):
        p = pxt[name]
        rnd1 = spool.tile([Q, BL], F32, tag=f"rnd1{name}")
        nc.vector.tensor_scalar_add(out=rnd1[:], in0=p[:], scalar1=MAGIC)
        rnd = spool.tile([Q, BL], F32, tag=f"rnd{name}")
        nc.vector.tensor_scalar_sub(out=rnd[:], in0=rnd1[:], scalar1=MAGIC)
        gm = spool.tile([Q, BL], F32, tag=f"gm{name}")
        nc.vector.tensor_tensor(out=gm[:], in0=rnd[:], in1=p[:], op=OP.is_gt)
        v0 = spool.tile([Q, BL], F32, tag=f"v0{name}")
        nc.vector.tensor_sub(out=v0[:], in0=rnd[:], in1=gm[:])
        v0c = spool.tile([Q, BL], F32, tag=f"v0c{name}")
        nc.vector.tensor_scalar(out=v0c[:], in0=v0[:], scalar1=float(NY - 2),
                                scalar2=0.0, op0=OP.min, op1=OP.max)
        w = spool.tile([Q, BL], F32, tag=f"w{name}")
        nc.vector.tensor_sub(out=w[:], in0=p[:], in1=v0c[:])
        pos0[name] = v0c
        wgt[name] = w

    # off0[q, b] = b*FS + y0*512 + x0  (exact in f32, max < 2^23)
    offa = spool.tile([Q, BL], F32)
    nc.vector.tensor_scalar_mul(out=offa[:], in0=pos0["y"][:],
                                scalar1=float(NY))
    offb = spool.tile([Q, BL], F32)
    nc.vector.tensor_add(out=offb[:], in0=offa[:], in1=pos0["x"][:])
    offc = spool.tile([Q, BL], F32)
    nc.vector.tensor_add(out=offc[:], in0=offb[:], in1=bbase_sb[:])
    offi = spool.tile([Q, BL], I32)
    nc.vector.tensor_copy(out=offi[:], in_=offc[:])

    # ---------------- per-sample gathers ----------------
    # One indirect DMA per sample: 64 partitions (queries), each fetching
    # a contiguous 514-float run that covers all 4 bilinear corners
    # (cols 0, 1, 512, 513).
    GW = 520  # padded run length per query
    field_flat = field_d[:].rearrange("b y x -> (b y x)")[:, None]
    G = spool.tile([Q, BL, GW], F32)
    for b in range(BL):
        nc.gpsimd.indirect_dma_start(
            out=G[:, b, 0:NY + 2], out_offset=None, in_=field_flat,
            in_offset=bass.IndirectOffsetOnAxis(ap=offi[:, b:b + 1], axis=0))

    # ---------------- bilinear combine (q-layout) ----------------
    def gcol(c):
        return G[:, :, c:c + 1].rearrange("q b o -> q (b o)")

    wx, wy = wgt["x"], wgt["y"]
    d0 = spool.tile([Q, BL], F32)
    nc.vector.tensor_sub(out=d0[:], in0=gcol(1), in1=gcol(0))
    m0 = spool.tile([Q, BL], F32)
    nc.vector.tensor_mul(out=m0[:], in0=d0[:], in1=wx[:])
    ex0 = spool.tile([Q, BL], F32)
    nc.vector.tensor_add(out=ex0[:], in0=gcol(0), in1=m0[:])
    d1 = spool.tile([Q, BL], F32)
    nc.vector.tensor_sub(out=d1[:], in0=gcol(NY + 1), in1=gcol(NY))
    m1 = spool.tile([Q, BL], F32)
    nc.vector.tensor_mul(out=m1[:], in0=d1[:], in1=wx[:])
    ex1 = spool.tile([Q, BL], F32)
    nc.vector.tensor_add(out=ex1[:], in0=gcol(NY), in1=m1[:])
    dy = spool.tile([Q, BL], F32)
    nc.vector.tensor_sub(out=dy[:], in0=ex1[:], in1=ex0[:])
    my = spool.tile([Q, BL], F32)
    nc.vector.tensor_mul(out=my[:], in0=dy[:], in1=wy[:])
    exy_q = spool.tile([Q, BL], F32)
    nc.vector.tensor_add(out=exy_q[:], in0=ex0[:], in1=my[:])

    # transpose back to sample-on-partition layout [BL, Q]
    exy_ps = psum.tile([BL, Q], F32, tag="tr")
    nc.tensor.transpose(out=exy_ps[:], in_=exy_q[:], identity=ident[0:Q, 0:Q])
    exy = exy_ps

    # ---------------- field statistics (the memory-bound stream) --------
    part_s = spool.tile([P, BL], F32)
    part_q = spool.tile([P, BL], F32)
    for t in range(BL // SPD):
        ft = fpool.tile([P, SPD * COLS], F32)
        nc.sync.dma_start(
            out=ft[:].rearrange("p (b a y) -> p b a y", b=SPD, a=NX // P,
                                y=NY),
            in_=field_d[t * SPD:(t + 1) * SPD].rearrange(
                "b (p a) y -> p b a y", p=P))
        for s in range(SPD):
            b = t * SPD + s
            nc.vector.reduce_sum(out=part_s[:, b:b + 1],
                                 in_=ft[:, s * COLS:(s + 1) * COLS], axis=AX.X)
            sq = scr.tile([P, COLS], F32, tag="sq")
            nc.scalar.activation(out=sq[:], in_=ft[:, s * COLS:(s + 1) * COLS],
                                 func=AF.Square, accum_out=part_q[:, b:b + 1])

    # cross-partition aggregation: PE transpose + free-dim reduce
    ts_ps = psum.tile([BL, P], F32, tag="tr")
    nc.tensor.transpose(out=ts_ps[:], in_=part_s[:], identity=ident[:])
    tq_ps = psum.tile([BL, P], F32, tag="tr")
    nc.tensor.transpose(out=tq_ps[:], in_=part_q[:], identity=ident[:])
    Ssum = spool.tile([BL, 1], F32)
    Qsum = spool.tile([BL, 1], F32)
    nc.vector.reduce_sum(out=Ssum[:], in_=ts_ps[:], axis=AX.X)
    nc.vector.reduce_sum(out=Qsum[:], in_=tq_ps[:], axis=AX.X)

    # mu = S/N ; var = (Q - S^2/N)/(N-1) ; sd = max(sqrt(var), 1e-6)
    mu = spool.tile([BL, 1], F32)
    nc.vector.tensor_scalar_mul(out=mu[:], in0=Ssum[:], scalar1=1.0 / FS)
    s2 = spool.tile([BL, 1], F32)
    nc.vector.tensor_mul(out=s2[:], in0=Ssum[:], in1=mu[:])
    varn = spool.tile([BL, 1], F32)
    nc.vector.tensor_sub(out=varn[:], in0=Qsum[:], in1=s2[:])
    var = spool.tile([BL, 1], F32)
    nc.vector.tensor_scalar_mul(out=var[:], in0=varn[:],
                                scalar1=1.0 / (FS - 1))
    sd = spool.tile([BL, 1], F32)
    nc.scalar.activation(out=sd[:], in_=var[:], func=AF.Sqrt)
    sdc = spool.tile([BL, 1], F32)
    nc.vector.tensor_scalar_max(out=sdc[:], in0=sd[:], scalar1=1e-6)
    inv = spool.tile([BL, 1], F32)
    nc.vector.reciprocal(out=inv[:], in_=sdc[:])
    nmi0 = spool.tile([BL, 1], F32)
    nc.vector.tensor_mul(out=nmi0[:], in0=mu[:], in1=inv[:])
    nmi = spool.tile([BL, 1], F32)
    nc.vector.tensor_scalar_mul(out=nmi[:], in0=nmi0[:], scalar1=-1.0)

    # pde = (exy - mu) / sd = exy*inv + (-mu*inv)
    pde = spool.tile([BL, Q], F32)
    nc.scalar.activation(out=pde[:], in_=exy[:], func=AF.Identity,
                         bias=nmi[:, 0:1], scale=inv[:, 0:1])

    # ---------------- einsum('bq,bqd->bd') on DVE ----------------
    prod = spool.tile([BL, Q * D], F32)
    nc.vector.tensor_tensor(
        out=prod[:].rearrange("p (q d) -> p q d", d=D),
        in0=pde[:].rearrange("p (q o) -> p q o", o=1).to_broadcast([BL, Q, D]),
        in1=qv[:, :, 2:CH], op=OP.mult)
    outt = spool.tile([BL, D], F32)
    nc.vector.reduce_sum(out=outt[:],
                         in_=prod[:].rearrange("p (q d) -> p d q", d=D),
                         axis=AX.X)
    nc.sync.dma_start(out=out_d[:], in_=outt[:])

    if dbg_d is not None:
        dbg = spool.tile([Q, DBGW], F32)
        nc.vector.tensor_copy(out=dbg[:, 0:32], in_=pxt["x"][:])
        nc.vector.tensor_copy(out=dbg[:, 32:64], in_=pxt["y"][:])
        nc.vector.tensor_copy(out=dbg[:, 64:96], in_=pos0["x"][:])
        nc.vector.tensor_copy(out=dbg[:, 96:128], in_=pos0["y"][:])
        nc.vector.tensor_copy(out=dbg[:, 128:160], in_=wgt["x"][:])
        nc.vector.tensor_copy(out=dbg[:, 160:192], in_=wgt["y"][:])
        nc.vector.tensor_copy(out=dbg[:, 192:224], in_=offc[:])
        nc.vector.tensor_copy(out=dbg[:, 224:256], in_=gcol(0))
        nc.vector.tensor_copy(out=dbg[:, 256:288], in_=gcol(1))
        nc.vector.tensor_copy(out=dbg[:, 288:320], in_=gcol(NY))
        nc.vector.tensor_copy(out=dbg[:, 320:352], in_=gcol(NY + 1))
        nc.vector.tensor_copy(out=dbg[:, 352:384], in_=exy_q[:])
        nc.vector.tensor_copy(out=dbg[0:BL, 384:448], in_=pde[:])
        nc.vector.tensor_copy(out=dbg[0:BL, 448:449], in_=mu[:])
        nc.vector.tensor_copy(out=dbg[0:BL, 449:450], in_=sdc[:])
        nc.vector.tensor_copy(out=dbg[0:BL, 450:451], in_=inv[:])
        nc.vector.tensor_copy(out=dbg[0:BL, 451:452], in_=Ssum[:])
        nc.vector.tensor_copy(out=dbg[0:BL, 452:453], in_=Qsum[:])
        nc.sync.dma_start(out=dbg_d[:], in_=dbg[:])


def build(debug_out: bool = DBG):
    nc = bacc.Bacc("TRN2", target_bir_lowering=False, debug=False,
                   num_devices=NCORES)
    meas_d = nc.dram_tensor("meas", [BL, S], F32, kind="ExternalInput").ap()
    field_d = nc.dram_tensor("field", [BL, NX, NY], F32,
                             kind="ExternalInput").ap()
    w1_d = nc.dram_tensor("w1", [S, H], F32, kind="ExternalInput").ap()
    b1_d = nc.dram_tensor("b1", [H], F32, kind="ExternalInput").ap()
    w2_d = nc.dram_tensor("w2", [H, Q * CH], F32, kind="ExternalInput").ap()
    b2_d = nc.dram_tensor("b2", [Q * CH], F32, kind="ExternalInput").ap()
    bbase_d = nc.dram_tensor("bbase", [Q, BL], F32, kind="ExternalInput").ap()
    out_d = nc.dram_tensor("out", [BL, D], F32, kind="ExternalOutput").ap()
    dbg_d = None
    if debug_out:
        dbg_d = nc.dram_tensor("dbg", [Q, DBGW], F32,
                               kind="ExternalOutput").ap()
    with tile.TileContext(nc) as tc:
        with ExitStack() as ctx:
            _body(ctx, tc, meas_d, field_d, w1_d, b1_d, w2_d, b2_d, bbase_d,
                  out_d, dbg_d)
    nc.compile()
    return nc


_CACHE = {}


def _get_nc():
    if "nc" not in _CACHE:
        _CACHE["nc"] = build()
    return _CACHE["nc"]


def make_in_maps(measurement, field_u, W1, b1, W2, b2):
    ms = np.ascontiguousarray(np.asarray(measurement, np.float32))
    fu = np.ascontiguousarray(np.asarray(field_u, np.float32))
    w1 = np.ascontiguousarray(np.asarray(W1, np.float32))
    b1a = np.ascontiguousarray(np.asarray(b1, np.float32))
    w2 = np.ascontiguousarray(np.asarray(W2, np.float32))
    b2a = np.ascontiguousarray(np.asarray(b2, np.float32))
    bbase = np.ascontiguousarray(
        np.broadcast_to((np.arange(BL, dtype=np.float32) * FS), (Q, BL)))
    in_maps = []
    for c in range(NCORES):
        sl = slice(c * BL, (c + 1) * BL)
        in_maps.append({
            "meas": np.ascontiguousarray(ms[sl]),
            "field": np.ascontiguousarray(fu[sl]),
            "w1": w1, "b1": b1a, "w2": w2, "b2": b2a, "bbase": bbase,
        })
    return in_maps


def kernel(measurement, field_u, W1, b1, W2, b2):
    nc = _get_nc()
    in_maps = make_in_maps(measurement, field_u, W1, b1, W2, b2)
    res = run_bass_kernel_spmd(nc, in_maps, core_ids=list(range(NCORES)))
    return np.concatenate([r["out"] for r in res.results], axis=0)
